# revision 1
# baseline (speedup 1.0000x reference)
"""Trainium2 Bass kernel for a 2-layer GENConv (softmax aggr) + LayerNorm GNN block.

Distribution: graph-partitioned across 8 NeuronCores. Nodes are reordered by a
Fiedler-vector (spectral 1D) layout so the adjacency becomes banded; the
per-channel softmax aggregation collapses to two banded-SpMM matmuls because
GENConv softmax logits depend only on the source node:

  r = relu(x); w = exp(t*r); q = w*r
  num = A @ q;  den = A @ w;  agg = num/den + eps     (exactly the reference
  softmax aggregation; the max-shift and the eps inside exp cancel)

Each core owns 4 contiguous dst blocks of 128 nodes; its banded A^T slab and
the qw window it contracts against are uniform across cores (SPMD), with
per-core variation expressed purely through input data (zero-padded bands).

Three SPMD launches: conv1 -> (host moves x1 slices) -> conv2+LN+colsums ->
(host stacks partials) -> finalize (tiny matvec -> row-0 update). The host does
no float arithmetic: only slicing/permutation/concatenation of device results.
"""

import ml_dtypes
import numpy as np

import concourse.bass as bass
import concourse.bacc as bacc
import concourse.mybir as mybir
import concourse.tile as tile
import concourse.masks as masks
from concourse.bass_utils import run_bass_kernel_spmd

F32 = mybir.dt.float32
BF16 = mybir.dt.bfloat16
AF = mybir.ActivationFunctionType
ALU = mybir.AluOpType

N_CORES = 8
H = 768
CHT = H // 128           # channel tiles = 6
EPS_MSG = 1e-7
LN_EPS = 1e-5

_cache = {}


# ----------------------------------------------------------------------------
# Host-side graph preprocessing (index work only — no float math on data).
# ----------------------------------------------------------------------------

def _ordering(src, dst, n):
    """1D spectral (Fiedler) layout of the graph; falls back to RCM/identity."""
    import scipy.sparse as sp
    a = sp.csr_matrix(
        (np.ones(len(src), dtype=np.float64), (dst, src)), shape=(n, n)
    )
    asym = ((a + a.T) > 0).astype(np.float64)
    try:
        from scipy.sparse.linalg import eigsh
        lap = sp.diags(np.asarray(asym.sum(1)).ravel()) - asym
        _, vecs = eigsh(lap, k=2, sigma=-1e-4, which="LM")
        return np.argsort(vecs[:, 1]).astype(np.int64)
    except Exception:
        try:
            from scipy.sparse.csgraph import reverse_cuthill_mckee
            return np.asarray(
                reverse_cuthill_mckee(asym.tocsr(), symmetric_mode=True)
            ).astype(np.int64)
        except Exception:
            return np.arange(n, dtype=np.int64)


def _prepare(edge_index, n):
    import scipy.sparse as sp
    src = np.asarray(edge_index[0], dtype=np.int64)
    dst = np.asarray(edge_index[1], dtype=np.int64)
    perm = _ordering(src, dst, n)           # new position i holds old node perm[i]
    inv = np.empty(n, dtype=np.int64)
    inv[perm] = np.arange(n)
    ns, nd = inv[src], inv[dst]             # edges in new coordinates

    nb = n // 128                           # dst blocks (128 nodes each)
    bpc = nb // N_CORES                     # blocks per core
    # global band extents (multiples of 128)
    pmax = qmax = 0
    order = np.lexsort((ns, nd))
    ns_s, nd_s = ns[order], nd[order]
    starts = np.searchsorted(nd_s, np.arange(0, n, 128))
    ends = np.searchsorted(nd_s, np.arange(128, n + 1, 128))
    for b in range(nb):
        s = ns_s[starts[b]:ends[b]]
        if len(s) == 0:
            continue
        lo = (s.min() // 128) * 128
        hi = ((s.max() // 128) + 1) * 128
        pmax = max(pmax, b * 128 - lo)
        qmax = max(qmax, hi - (b + 1) * 128)
    t_tiles = (pmax + 128 + qmax) // 128    # contraction tiles per dst block
    wx = bpc * 128 + pmax + qmax            # per-core source-window rows

    # banded A^T slabs, packed partition-major for contiguous DMA:
    # ab[c][p, (bl*T + t)*128 + d] = #edges src=(blk-pmax+t*128+p) -> dst=(blk+d)
    amat = sp.csr_matrix(
        (np.ones(len(ns), dtype=np.float64), (nd, ns)), shape=(n, n))
    abands = []
    for c in range(N_CORES):
        ab = np.zeros((128, bpc * t_tiles * 128), dtype=np.float32)
        for bl in range(bpc):
            blk = (c * bpc + bl) * 128
            w0 = blk - pmax
            sub = amat[blk:blk + 128, max(w0, 0):min(w0 + t_tiles * 128, n)]
            subd = np.asarray(sub.todense(), dtype=np.float32)  # [128 dst, win]
            j0 = max(w0, 0) - w0
            at = np.zeros((t_tiles * 128, 128), dtype=np.float32)
            at[j0:j0 + subd.shape[1], :] = subd.T
            for t in range(t_tiles):
                ab[:, (bl * t_tiles + t) * 128:(bl * t_tiles + t + 1) * 128] = \
                    at[t * 128:(t + 1) * 128, :]
        abands.append(ab.astype(ml_dtypes.bfloat16))

    return dict(perm=perm, inv=inv, pmax=pmax, qmax=qmax, t=t_tiles, wx=wx,
                bpc=bpc, abands=abands)


def _win_slice(full, c, bpc, pmax, qmax):
    """rows [c*bpc*128 - pmax, (c+1)*bpc*128 + qmax) of `full`, zero-padded."""
    n = full.shape[0]
    lo = c * bpc * 128 - pmax
    hi = (c + 1) * bpc * 128 + qmax
    out = np.zeros((hi - lo, full.shape[1]), dtype=full.dtype)
    a, b = max(lo, 0), min(hi, n)
    out[a - lo:b - lo] = full[a:b]
    return out


# ----------------------------------------------------------------------------
# Bass programs.
# ----------------------------------------------------------------------------

def _build_conv2(prep, with_tail, enable_ln=True, enable_cs=True):
    """One GENConv layer. with_tail=False: outputs x_out [bpc*128, H] (the new
    node features). with_tail=True: additionally applies LayerNorm+relu
    (DeepGCN 'res' block) and outputs per-core column sums of x1 and x2 as
    csout [1, 2*H]. enable_ln/enable_cs are debug bisection switches."""
    t_tiles, wx, bpc = prep["t"], prep["wx"], prep["bpc"]
    own_off = prep["pmax"] // 128           # xin tile index of first own block
    nc = bacc.Bacc("TRN2", target_bir_lowering=False, debug=False,
                   enable_asserts=False, num_devices=N_CORES)
    nxt = wx // 128                         # source-window tiles
    xin = nc.dram_tensor("xin", [wx, H], F32, kind="ExternalInput")
    ab = nc.dram_tensor("ab", [128, bpc * t_tiles * 128], BF16, kind="ExternalInput")
    wt = nc.dram_tensor("wt", [128, CHT * H], BF16, kind="ExternalInput")
    br = nc.dram_tensor("br", [128, H], F32, kind="ExternalInput")
    ts = nc.dram_tensor("ts", [128, 1], F32, kind="ExternalInput")
    if with_tail:
        lngr = nc.dram_tensor("lngr", [128, H], F32, kind="ExternalInput")
        lnbr = nc.dram_tensor("lnbr", [128, H], F32, kind="ExternalInput")
        csout = nc.dram_tensor("csout", [1, 2 * H], F32, kind="ExternalOutput")
    else:
        xout = nc.dram_tensor("xout", [bpc * 128, H], F32, kind="ExternalOutput")

    xin_r = xin.rearrange("(n p) d -> n p d", p=128)
    if not with_tail:
        xout_r = xout.rearrange("(n p) d -> n p d", p=128)

    with tile.TileContext(nc) as tc:
        with (
            tc.tile_pool(name="persist", bufs=1) as pp,
            tc.tile_pool(name="stream", bufs=3) as sp_pool,
            tc.tile_pool(name="epi", bufs=1) as ep,
            tc.tile_pool(name="psA", bufs=1, space="PSUM") as psA,
            tc.tile_pool(name="psT", bufs=2, space="PSUM") as psT,
            tc.tile_pool(name="psC", bufs=1, space="PSUM") as psC,
        ):
            # persistent tiles
            qw = pp.tile([128, nxt * 2 * H], BF16)       # [q | w] per window tile
            wt_sb = pp.tile([128, CHT * H], BF16)
            ab_sb = pp.tile([128, bpc * t_tiles * 128], BF16)
            nc.sync.dma_start(ab_sb[:], ab[:])
            br_sb = pp.tile([128, H], F32)
            ts_sb = pp.tile([128, 1], F32)
            ident = pp.tile([128, 128], F32)
            eps_sb = pp.tile([128, 1], F32)
            nc.gpsimd.memset(eps_sb[:], EPS_MSG)
            masks.make_identity(nc, ident[:])
            nc.sync.dma_start(wt_sb[:], wt[:])
            nc.sync.dma_start(br_sb[:], br[:])
            nc.sync.dma_start(ts_sb[:], ts[:])
            if with_tail:
                lng_sb = pp.tile([128, H], F32)
                lnb_sb = pp.tile([128, H], F32)
                ones_sb = pp.tile([128, 1], F32)
                cs_ps = psC.tile([1, 2 * H], F32)
                lneps_sb = pp.tile([128, 1], F32)
                nc.gpsimd.memset(lneps_sb[:], LN_EPS)
                nc.sync.dma_start(lng_sb[:], lngr[:])
                nc.sync.dma_start(lnb_sb[:], lnbr[:])
                nc.gpsimd.memset(ones_sb[:], 1.0)

            # per-node pass over the source window: r -> w = exp(t*r) -> q = w*r
            for s in range(nxt):
                qs = qw[:, 2 * s * H:(2 * s + 1) * H]
                ws = qw[:, (2 * s + 1) * H:(2 * s + 2) * H]
                xt = sp_pool.tile([128, H], F32, tag="xt")
                nc.sync.dma_start(xt[:], xin_r[s])
                nc.vector.tensor_scalar_max(qs, xt[:], 0.0)
                nc.scalar.activation(ws, qs, AF.Exp, scale=ts_sb[:, 0:1])
                nc.vector.tensor_mul(qs, qs, ws)

            # per dst-block: banded SpMM -> softmax divide -> +x_own -> W matmul
            for bl in range(bpc):
                agg = psA.tile([128, 2 * H], F32, tag="agg")
                for t in range(t_tiles):
                    at = ab_sb[:, (bl * t_tiles + t) * 128:(bl * t_tiles + t + 1) * 128]
                    qwrow = bl + t          # window tile for this contraction
                    for ch in range(3):     # 1536 free = 3 x 512
                        nc.tensor.matmul(
                            agg[:, ch * 512:(ch + 1) * 512],
                            at,
                            qw[:, 2 * qwrow * H + ch * 512:2 * qwrow * H + (ch + 1) * 512],
                            start=(t == 0), stop=(t == t_tiles - 1),
                        )
                # own rows of xin for this block (residual input h)
                xo = ep.tile([128, H], F32, tag="xo")
                nc.sync.dma_start(xo[:], xin_r[own_off + bl])

                m = ep.tile([128, H], F32, tag="m")
                rec = ep.tile([128, H], F32, tag="rec")
                nc.vector.reciprocal_approx_accurate(rec[:], agg[:, H:2 * H], m[:])
                nc.vector.tensor_mul(m[:], agg[:, 0:H], rec[:])
                nc.scalar.activation(m[:], m[:], AF.Identity, bias=eps_sb[:, 0:1])
                nc.vector.tensor_add(m[:], m[:], xo[:])

                # transpose M -> lhsT tiles, then x_new = M @ W.T + b
                mt = ep.tile([128, H], BF16, tag="mt")
                for c in range(CHT):
                    tp = psT.tile([128, 128], F32, tag="tp")
                    nc.tensor.transpose(tp[:], m[:, c * 128:(c + 1) * 128], ident[:])
                    nc.scalar.copy(mt[:, c * 128:(c + 1) * 128], tp[:])
                xps = psA.tile([128, H], F32, tag="agg")
                for c in range(CHT):
                    nc.tensor.matmul(
                        xps[:, 0:512], mt[:, c * 128:(c + 1) * 128],
                        wt_sb[:, c * H:c * H + 512],
                        start=(c == 0), stop=(c == CHT - 1))
                    nc.tensor.matmul(
                        xps[:, 512:H], mt[:, c * 128:(c + 1) * 128],
                        wt_sb[:, c * H + 512:(c + 1) * H],
                        start=(c == 0), stop=(c == CHT - 1))
                xn = ep.tile([128, H], F32, tag="xn")
                nc.vector.tensor_add(xn[:], xps[:], br_sb[:])

                if not with_tail:
                    nc.sync.dma_start(xout_r[bl], xn[:])
                elif not enable_ln:
                    x2 = ep.tile([128, H], F32, tag="x2")
                    nc.vector.tensor_add(x2[:], xn[:], xo[:])
                    if enable_cs:
                        first, last = bl == 0, bl == bpc - 1
                        nc.tensor.matmul(cs_ps[:, 0:512], ones_sb[:], xo[:, 0:512],
                                         start=first, stop=last)
                        nc.tensor.matmul(cs_ps[:, 512:768], ones_sb[:], xo[:, 512:768],
                                         start=first, stop=False)
                        nc.tensor.matmul(cs_ps[:, 768:1024], ones_sb[:], x2[:, 0:256],
                                         start=False, stop=last)
                        nc.tensor.matmul(cs_ps[:, 1024:1536], ones_sb[:], x2[:, 256:768],
                                         start=first, stop=last)
                else:
                    # LayerNorm over channels (free axis), then relu, then +x1
                    sm = ep.tile([128, 1], F32, tag="sm")
                    ssq = ep.tile([128, 1], F32, tag="ssq")
                    sqs = ep.tile([128, H], F32, tag="sqs")
                    nc.vector.tensor_reduce(sm[:], xn[:], mybir.AxisListType.X, ALU.add)
                    nc.vector.tensor_mul(sqs[:], xn[:], xn[:])
                    nc.vector.tensor_reduce(ssq[:], sqs[:], mybir.AxisListType.X, ALU.add)
                    mu = ep.tile([128, 1], F32, tag="mu")
                    var = ep.tile([128, 1], F32, tag="var")
                    nc.vector.tensor_scalar_mul(mu[:], sm[:], 1.0 / H)
                    nc.vector.tensor_scalar_mul(var[:], ssq[:], 1.0 / H)
                    mu2 = ep.tile([128, 1], F32, tag="mu2")
                    nc.vector.tensor_mul(mu2[:], mu[:], mu[:])
                    nc.vector.tensor_sub(var[:], var[:], mu2[:])
                    # rstd = sqrt(1/(var+eps)) — ACT Sqrt only (no table thrash)
                    rstd = ep.tile([128, 1], F32, tag="rstd")
                    rscr = ep.tile([128, 1], F32, tag="rscr")
                    nc.vector.tensor_scalar(var[:], var[:], lneps_sb[:, 0:1], None, ALU.add)
                    nc.vector.reciprocal_approx_accurate(rstd[:], var[:], rscr[:])
                    nc.scalar.sqrt(rstd[:], rstd[:])
                    nmr = ep.tile([128, 1], F32, tag="nmr")
                    nc.vector.tensor_mul(nmr[:], mu[:], rstd[:])
                    nc.vector.tensor_scalar_mul(nmr[:], nmr[:], -1.0)
                    hn = ep.tile([128, H], F32, tag="hn")
                    nc.scalar.activation(hn[:], xn[:], AF.Identity,
                                         bias=nmr[:, 0:1], scale=rstd[:, 0:1])
                    nc.vector.tensor_mul(hn[:], hn[:], lng_sb[:])
                    nc.vector.tensor_add(hn[:], hn[:], lnb_sb[:])
                    nc.scalar.activation(hn[:], hn[:], AF.Relu)
                    x2 = ep.tile([128, H], F32, tag="x2")
                    nc.vector.tensor_add(x2[:], hn[:], xo[:])
                    # column sums as a [1, 2H] row: one PSUM group per 2KB bank
                    # spanning all blocks (x1 cols 0:768 = xo, x2 cols 768:1536).
                    if enable_cs:
                        first, last = bl == 0, bl == bpc - 1
                        nc.tensor.matmul(cs_ps[:, 0:512], ones_sb[:], xo[:, 0:512],
                                         start=first, stop=last)
                        nc.tensor.matmul(cs_ps[:, 512:768], ones_sb[:], xo[:, 512:768],
                                         start=first, stop=False)
                        nc.tensor.matmul(cs_ps[:, 768:1024], ones_sb[:], x2[:, 0:256],
                                         start=False, stop=last)
                        nc.tensor.matmul(cs_ps[:, 1024:1536], ones_sb[:], x2[:, 256:768],
                                         start=first, stop=last)
            if with_tail:
                cs_sb = pp.tile([1, 2 * H], F32)
                if enable_cs:
                    nc.scalar.copy(cs_sb[:], cs_ps[:])
                else:
                    nc.gpsimd.memset(cs_sb[:], 0.0)
                nc.sync.dma_start(csout[:], cs_sb[:])
    nc.compile()
    return nc


def _build_final(n):
    """Sum per-core colsum partials, matvec through Wc, add bc and x[0]."""
    nc = bacc.Bacc("TRN2", target_bir_lowering=False, debug=False,
                   enable_asserts=False, num_devices=N_CORES)
    parts = nc.dram_tensor("parts", [128, N_CORES * 2 * CHT], F32, kind="ExternalInput")
    wct = nc.dram_tensor("wct", [128, 2 * CHT * H], F32, kind="ExternalInput")
    bcr = nc.dram_tensor("bcr", [1, H], F32, kind="ExternalInput")
    x0r = nc.dram_tensor("x0r", [1, H], F32, kind="ExternalInput")
    row0 = nc.dram_tensor("row0", [1, H], F32, kind="ExternalOutput")

    with tile.TileContext(nc) as tc:
        with (
            tc.tile_pool(name="sb", bufs=1) as sb,
            tc.tile_pool(name="ps", bufs=1, space="PSUM") as ps,
        ):
            # parts is channel-major [128, (core, 2CHT)]; reduce over cores
            pt = sb.tile([128, N_CORES * 2 * CHT], F32)
            nc.sync.dma_start(pt[:], parts[:])
            acc = sb.tile([128, 2 * CHT], F32)
            nc.vector.tensor_reduce(
                acc[:], pt[:].rearrange("p (a d) -> p d a", a=N_CORES),
                mybir.AxisListType.X, ALU.add)
            nc.vector.tensor_scalar_mul(acc[:], acc[:], 1.0 / n)

            wct_sb = sb.tile([128, 2 * CHT * H], F32)
            nc.sync.dma_start(wct_sb[:], wct[:])
            g_ps = ps.tile([1, H], F32)
            for j in range(2 * CHT):
                for lo, hi in ((0, 512), (512, H)):   # per-bank chunks
                    nc.tensor.matmul(
                        g_ps[:, lo:hi],
                        acc[:, j:j + 1],
                        wct_sb[:, j * H + lo:j * H + hi],
                        start=(j == 0), stop=(j == 2 * CHT - 1))
            bc_sb = sb.tile([1, H], F32)
            x0_sb = sb.tile([1, H], F32)
            out_sb = sb.tile([1, H], F32)
            nc.sync.dma_start(bc_sb[:], bcr[:])
            nc.sync.dma_start(x0_sb[:], x0r[:])
            nc.vector.tensor_add(out_sb[:], g_ps[:], bc_sb[:])
            nc.vector.tensor_add(out_sb[:], out_sb[:], x0_sb[:])
            nc.sync.dma_start(row0[:], out_sb[:])
    nc.compile()
    return nc


def _pack_wt(w, dtype=np.float32):
    """[Hout, Hin] weight -> partition-major packed W.T tiles [128, (Hin/128)*Hout]:
    out[p, c*Hout + o] = W[o, c*128 + p]"""
    h_out, h_in = w.shape
    nt = h_in // 128
    out = np.empty((128, nt * h_out), dtype=np.float32)
    for c in range(nt):
        out[:, c * h_out:(c + 1) * h_out] = w[:, c * 128:(c + 1) * 128].T
    return np.ascontiguousarray(out.astype(dtype))


def kernel(**inputs):
    x = np.asarray(inputs["x"], dtype=np.float32)
    w1 = np.asarray(inputs["W1"], dtype=np.float32)
    b1 = np.asarray(inputs["b1"], dtype=np.float32)
    t1 = np.float32(np.asarray(inputs["t1"]))
    w2 = np.asarray(inputs["W2"], dtype=np.float32)
    b2 = np.asarray(inputs["b2"], dtype=np.float32)
    t2 = np.float32(np.asarray(inputs["t2"]))
    ln_g = np.asarray(inputs["ln_g"], dtype=np.float32)
    ln_b = np.asarray(inputs["ln_b"], dtype=np.float32)
    wc = np.asarray(inputs["Wc"], dtype=np.float32)
    bc = np.asarray(inputs["bc"], dtype=np.float32)
    ei = np.asarray(inputs["edge_index"])

    n = x.shape[1]
    ekey = (ei.shape[1], n,
            int(np.bitwise_xor.reduce(ei[0].astype(np.int64) * 31 + ei[1])))
    if ekey not in _cache:
        prep = _prepare(ei, n)
        progs = dict(
            conv=_build_conv2(prep, False),
            tail=_build_conv2(prep, True),
            fin=_build_final(n),
        )
        _cache[ekey] = (prep, progs)
    prep, progs = _cache[ekey]
    perm, pmax, qmax, bpc = prep["perm"], prep["pmax"], prep["qmax"], prep["bpc"]

    xp = np.ascontiguousarray(x[0][perm])            # permuted node features
    t1r = np.full((128, 1), t1, dtype=np.float32)
    t2r = np.full((128, 1), t2, dtype=np.float32)
    w1t, w2t = _pack_wt(w1, ml_dtypes.bfloat16), _pack_wt(w2, ml_dtypes.bfloat16)
    b1r = np.ascontiguousarray(np.broadcast_to(b1, (128, H)))
    b2r = np.ascontiguousarray(np.broadcast_to(b2, (128, H)))
    lngr = np.ascontiguousarray(np.broadcast_to(ln_g, (128, H)))
    lnbr = np.ascontiguousarray(np.broadcast_to(ln_b, (128, H)))

    cores = list(range(N_CORES))

    # --- launch 1: conv1 ---
    maps1 = [dict(xin=_win_slice(xp, c, bpc, pmax, qmax), ab=prep["abands"][c],
                  wt=w1t, br=b1r, ts=t1r) for c in cores]
    res1 = run_bass_kernel_spmd(progs["conv"], maps1, core_ids=cores)
    x1 = np.concatenate([res1.results[c]["xout"] for c in cores], axis=0)

    # --- launch 2: conv2 + LN + colsums ---
    maps2 = [dict(xin=_win_slice(x1, c, bpc, pmax, qmax), ab=prep["abands"][c],
                  wt=w2t, br=b2r, ts=t2r, lngr=lngr, lnbr=lnbr) for c in cores]
    res2 = run_bass_kernel_spmd(progs["tail"], maps2, core_ids=cores)
    # host shuffle (pure data movement): per-core [1, 2H] row -> channel-major
    # [128, 2CHT], stacked along free dim -> [128, cores*2CHT]
    parts = np.concatenate(
        [res2.results[c]["csout"].reshape(2 * CHT, 128).T for c in cores], axis=1)
    parts = np.ascontiguousarray(parts)

    # --- launch 3: finalize row 0 ---
    maps3 = [dict(parts=parts, wct=_pack_wt(wc),
                  bcr=bc.reshape(1, H).astype(np.float32),
                  x0r=np.ascontiguousarray(x[0, 0:1, :])) for _ in cores]
    res3 = run_bass_kernel_spmd(progs["fin"], maps3, core_ids=cores)
    row0 = res3.results[0]["row0"][0]

    out = x.copy()
    out[0, 0, :] = row0
    return out

